# revision 5
# baseline (speedup 1.0000x reference)
"""MACCL loss kernel for Trainium2 (8 NeuronCores, SPMD data-parallel).

Strategy (v2)
-------------
The O(B^2 D) contrastive part dominates (B=8192, D=256).  The host does
the O(B*D) data prep that used to run on-device (and was the pipeline
bottleneck): permute rows label-0-first, compute row norms, quantize the
transposed features to fp8(e4m3) in the [K=128, 2, B] DoubleRow layout.
Each core then only runs the O(B^2) part:

  - 8x 1024-wide fp8 DoubleRow matmuls per 2048-column group: one
    instruction contracts the full K=256 (two k-tiles packed), so the
    PE does 0.5 cyc/row and the whole sim matrix costs ~14us/core.
  - ACT exp(scale_i * psum) in place, scale = r_i/T as a per-partition
    AP (the row normalization folds into the activation for free), with
    accum_out producing per-label-segment row sums (columns are
    label-sorted, segments are contiguous).
  - The diagonal term is reproduced bit-exactly by a DR matmul over the
    core's own (lhsT unscaled, rhs r_j-scaled) fp8 slices -- the same
    host arrays the main loop consumes -- so pos_sum = S_same - d
    cancels exactly on the host.

Per-core output: stats [128, 24] = {S0, S1, exp(diag)} x 8 row tiles.
Host finalizes in f64 (center/margin/sigma/log/mean) exactly mirroring
the reference formulas; norms/rowsums for the center and sigma terms
are host-side f64 (more accurate than the reference's own f32).

The operand quantization error only touches r_con, which is ~0.3% of
the total loss, so fp8 keeps the end-to-end error ~1e-5.
"""

import os
import sys

for _p in ("/root/.axon_site", "/root/.axon_site/_ro/trn_rl_repo",
           "/root/.axon_site/_ro/pypackages", "/opt/trn_rl_repo", "/opt/pypackages"):
    if os.path.isdir(_p) and _p not in sys.path:
        sys.path.append(_p)

import numpy as np
import ml_dtypes
from contextlib import ExitStack

import concourse.bass as bass
import concourse.bacc as bacc
import concourse.tile as tile
from concourse import mybir
from concourse.bass_utils import run_bass_kernel_spmd

F32 = mybir.dt.float32
BF16 = mybir.dt.bfloat16
F8 = mybir.dt.float8e4

P = 128
D = 256
B = 8192
NCORES = 8
BPC = B // NCORES
MROW = BPC // P
GW = 2048
NG = B // GW
TEMPERATURE = 0.07
MARGIN_BASE = 0.5
LAMBDA_SIGMA = 0.3
LAMBDA_RESOLUTION = 0.3
RESOLUTION_RATIO = 224.0 / 900.0
ALPHA, BETA, GAMMA = 1.0, 1.0, 0.5

# "f8dr" = fp8 DoubleRow (1 matmul per chunk, 0.5 cyc/row)
# "bf16" = bf16 with explicit k-loop (fallback)
MODE = os.environ.get("MACCL_MODE", "f8dr")
# moving-operand width per matmul (psum bank = 512 f32)
N_MM = int(os.environ.get("MACCL_N_MM", "1024"))


def _segment_ranges(n0, gw=GW):
    """Column ranges per gw-wide group, split at the label boundary n0."""
    ranges = []
    for g in range(B // gw):
        lo, hi = g * gw, (g + 1) * gw
        cuts = sorted({lo, hi, min(max(n0, lo), hi)})
        for s, e in zip(cuts, cuts[1:]):
            if e > s:
                ranges.append((g, s, e, 0 if e <= n0 else 1))
    k0 = sum(1 for r in ranges if r[3] == 0)
    return ranges, k0


def build_program(n0, mode=MODE):
    use_f8 = mode == "f8dr"
    op_dt = F8 if use_f8 else BF16
    perf = mybir.MatmulPerfMode.DoubleRow if use_f8 else None

    ranges, k0 = _segment_ranges(n0)
    nslots = len(ranges)
    k1 = nslots - k0

    AX = mybir.AxisListType.X
    MUL = mybir.AluOpType.mult
    AF = mybir.ActivationFunctionType

    nc = bacc.Bacc("TRN2", target_bir_lowering=False, debug=False,
                   num_devices=NCORES)
    a8_d = nc.dram_tensor("a8", [P, 2, B], op_dt, kind="ExternalInput").ap()
    m8_d = nc.dram_tensor("m8", [P, 2, BPC], op_dt, kind="ExternalInput").ap()
    ms8_d = nc.dram_tensor("ms8", [P, 2, BPC], op_dt, kind="ExternalInput").ap()
    rot_d = nc.dram_tensor("rot", [P, MROW], F32, kind="ExternalInput").ap()
    ident_d = nc.dram_tensor("ident", [P, P], F32, kind="ExternalInput").ap()
    stats_d = nc.dram_tensor("stats", [P, 3 * MROW], F32, kind="ExternalOutput").ap()

    with tile.TileContext(nc) as tc, ExitStack() as ctx:
        singles = ctx.enter_context(tc.tile_pool(name="singles", bufs=1))
        scr_pool = ctx.enter_context(tc.tile_pool(name="scr", bufs=2))
        acc_pool = ctx.enter_context(tc.tile_pool(name="acc", bufs=MROW))
        ps_pool = ctx.enter_context(tc.tile_pool(name="ps", bufs=2, space="PSUM"))

        a8_sb = singles.tile([P, 2, B], op_dt)
        m8_sb = singles.tile([P, 2, BPC], op_dt)
        ms8_sb = singles.tile([P, 2, BPC], op_dt)
        rot_sb = singles.tile([P, MROW], F32)
        ident_t = singles.tile([P, P], F32)
        stats_sb = singles.tile([P, 3 * MROW], F32)

        nc.sync.dma_start(rot_sb, rot_d)
        nc.sync.dma_start(ident_t, ident_d)
        nc.sync.dma_start(m8_sb, m8_d)
        nc.sync.dma_start(ms8_sb, ms8_d)
        # column-group chunks so group-0 matmuls start early
        for g in range(NG):
            nc.sync.dma_start(a8_sb[:, :, g * GW:(g + 1) * GW],
                              a8_d[:, :, g * GW:(g + 1) * GW])

        # Priming activation: hoists the ~1.3us ACT table load into the DMA
        # lead-in (it otherwise sits right before the first real EXP).
        prime = scr_pool.tile([P, 1], F32, tag="prime")
        nc.scalar.activation(prime, rot_sb[:, 0:1], AF.Exp, scale=0.0)

        def mm(out_ap, lhsT, rhs_tile, c0, n):
            if use_f8:
                nc.tensor.matmul(out_ap, lhsT, rhs_tile[:, :, c0:c0 + n],
                                 start=True, stop=True, perf_mode=perf)
            else:
                for k in (0, 1):
                    nc.tensor.matmul(out_ap, lhsT[:, k, :],
                                     rhs_tile[:, k, c0:c0 + n],
                                     start=(k == 0), stop=(k == 1))

        accs = [acc_pool.tile([P, nslots], F32, tag="acc", name=f"acc{m}")
                for m in range(MROW)]

        # ---- main loop: per column group, per own row tile; the exact
        # diagonal terms are interleaved into group 0 so the PE fills ACT's
        # pipeline from t=0 with no separate diag phase ----
        for g in range(NG):
            for m in range(MROW):
                psg = ps_pool.tile([P, GW], F32, tag="ps")
                lhsT = m8_sb[:, :, m * P:(m + 1) * P]
                for s in range(GW // N_MM):
                    mm(psg[:, s * N_MM:(s + 1) * N_MM], lhsT, a8_sb,
                       g * GW + s * N_MM, N_MM)
                for slot, (gg, s, e, _lab) in enumerate(ranges):
                    if gg != g:
                        continue
                    rs, re = s - g * GW, e - g * GW
                    nc.scalar.activation(
                        psg[:, rs:re], psg[:, rs:re], AF.Exp,
                        scale=rot_sb[:, m:m + 1],
                        accum_out=accs[m][:, slot:slot + 1])
                if g == 0:
                    psd = ps_pool.tile([P, GW], F32, tag="ps", name=f"psd{m}")
                    for s in range(BPC // N_MM):
                        mm(psd[:, s * N_MM:(s + 1) * N_MM], lhsT, ms8_sb,
                           s * N_MM, N_MM)
                    off = m * P
                    nc.scalar.activation(psd[:, off:off + P],
                                         psd[:, off:off + P],
                                         AF.Exp, scale=rot_sb[:, m:m + 1])
                    scrd = scr_pool.tile([P, P], F32, tag="scr",
                                         name=f"scrd{m}")
                    nc.vector.scalar_tensor_tensor(
                        out=scrd, in0=psd[:, off:off + P], scalar=1.0,
                        in1=ident_t, op0=MUL, op1=MUL,
                        accum_out=stats_sb[:, 2 * MROW + m:2 * MROW + m + 1])

        # ---- per-row-tile S0/S1 ----
        for m in range(MROW):
            s0 = stats_sb[:, m:m + 1]
            s1 = stats_sb[:, MROW + m:MROW + m + 1]
            if k0 > 0:
                nc.vector.reduce_sum(s0, accs[m][:, 0:k0], axis=AX)
            else:
                nc.vector.memset(s0, 0.0)
            if k1 > 0:
                nc.vector.reduce_sum(s1, accs[m][:, k0:nslots], axis=AX)
            else:
                nc.vector.memset(s1, 0.0)

        nc.sync.dma_start(stats_d, stats_sb)

    nc.compile()
    return nc


_PROGRAM_CACHE = {}


def _get_program(n0):
    key = (n0, MODE, N_MM)
    if key not in _PROGRAM_CACHE:
        _PROGRAM_CACHE[key] = build_program(n0)
    return _PROGRAM_CACHE[key]


def run_device(features, labels, trace=False):
    """Host prep + 8-core device run.  Returns (stats dict aligned to the
    label-sorted permutation, permutation order, n0, raw results)."""
    Bq, d = features.shape
    assert d == D and Bq == B

    order = np.argsort(labels, kind="stable")
    n0 = int((labels == 0).sum())
    fp = np.ascontiguousarray(features[order]).astype(np.float32, copy=False)

    # host-side O(B*D) prep
    fp64 = fp.astype(np.float64)
    norms2 = (fp64 * fp64).sum(axis=1)                  # [B]
    rowsum = fp64.sum(axis=1)                           # [B]
    r = 1.0 / np.maximum(np.sqrt(norms2), 1e-12)        # [B]
    r32 = r.astype(np.float32)

    op_np = ml_dtypes.float8_e4m3 if MODE == "f8dr" else ml_dtypes.bfloat16
    # [K=128, 2, B] DoubleRow layout: D index = ktile*128 + partition
    fT = np.ascontiguousarray(fp.T).reshape(2, P, B).transpose(1, 0, 2)
    m8_full = np.ascontiguousarray(fT).astype(op_np)                 # unscaled
    a8 = np.ascontiguousarray(fT * r32[None, None, :]).astype(op_np)  # scaled

    rot_full = (r32 / np.float32(TEMPERATURE)).astype(np.float32)
    ident = np.eye(P, dtype=np.float32)

    nc = _get_program(n0)
    in_maps = []
    for c in range(NCORES):
        sl = slice(c * BPC, (c + 1) * BPC)
        in_maps.append({
            "a8": a8,
            "m8": np.ascontiguousarray(m8_full[:, :, sl]),
            "ms8": np.ascontiguousarray(a8[:, :, sl]),
            "rot": np.ascontiguousarray(
                rot_full[sl].reshape(MROW, P).T),
            "ident": ident,
        })
    res = run_bass_kernel_spmd(nc, in_maps, list(range(NCORES)), trace=trace)

    parts = []
    for c in range(NCORES):
        st = res.results[c]["stats"]          # [128, 3*MROW]
        arr = st.reshape(P, 3, MROW).transpose(1, 2, 0).reshape(3, BPC)
        parts.append(arr)
    full = np.concatenate(parts, axis=1)      # [3, B] in permuted row order
    stats = {"norms2": norms2, "rowsum": rowsum,
             "S0": full[0], "S1": full[1], "d": full[2]}
    return stats, order, n0, res


def finalize(stats, order, n0, labels, normal_center, running_sigma, B):
    """Host O(B) finalization mirroring the reference formulas (float64)."""
    labels_p = labels[order]
    nmf = (labels_p == 0)
    amf = (labels_p == 1)
    norms2 = stats["norms2"].astype(np.float64)
    rowsum = stats["rowsum"].astype(np.float64)
    S0 = stats["S0"].astype(np.float64)
    S1 = stats["S1"].astype(np.float64)
    ddiag = stats["d"].astype(np.float64)

    c = np.asarray(normal_center, dtype=np.float64)
    csq = float((c * c).sum())
    if csq != 0.0:
        raise NotImplementedError  # caller routes to the general-center path
    dist_sq = norms2  # center == 0
    n_normal = float(nmf.sum())

    with np.errstate(divide="ignore", invalid="ignore"):
        n_el = n_normal * D
        masked_sum = float((rowsum * nmf).sum())
        mean = masked_sum / n_el
        sum_sq_m = float((norms2 * nmf).sum())
        var = (sum_sq_m - 2.0 * mean * masked_sum + mean * mean * n_el) / (n_el - 1.0)
        sigma_new = 0.9 * float(running_sigma) + 0.1 * np.sqrt(var)

        m_adaptive = (MARGIN_BASE + LAMBDA_SIGMA * sigma_new
                      + LAMBDA_RESOLUTION * (1.0 - RESOLUTION_RATIO))
        dist = np.sqrt(np.maximum(dist_sq, 0.0))
        r_center = dist_sq * nmf
        r_margin = np.maximum(m_adaptive - dist, 0.0) * amf

        S_same = np.where(nmf, S0, S1)
        S_diff = np.where(nmf, S1, S0)
        pos_sum = S_same - ddiag
        neg_sum = S_diff
        n1 = B - n0
        cnt_pos = np.where(nmf, n0 - 1, n1 - 1)
        cnt_neg = np.where(nmf, n1, n0)
        has_both = (cnt_pos > 0) & (cnt_neg > 0)
        pos_safe = np.where(has_both, np.maximum(pos_sum, 1e-12), 1.0)
        den_safe = np.where(has_both, pos_sum + neg_sum + 1e-8, 1.0)
        r_con = np.where(has_both, -np.log(pos_safe / den_safe), 0.0)

        raw_total = ALPHA * r_center + BETA * r_margin + GAMMA * r_con
        total = raw_total.mean()
    return np.array(total, dtype=np.float32)


def _finalize_general_center(stats, order, n0, labels, normal_center,
                             running_sigma, B, features):
    """Fallback for a nonzero normal_center (not hit for spec inputs)."""
    labels_p = labels[order]
    fp = features[order].astype(np.float64)
    c = np.asarray(normal_center, dtype=np.float64)
    qc = fp @ c
    norms2 = stats["norms2"].astype(np.float64)
    dist_sq = norms2 - 2.0 * qc + float((c * c).sum())
    nmf = (labels_p == 0)
    amf = (labels_p == 1)
    rowsum = stats["rowsum"].astype(np.float64)
    S0 = stats["S0"].astype(np.float64)
    S1 = stats["S1"].astype(np.float64)
    ddiag = stats["d"].astype(np.float64)
    n_normal = float(nmf.sum())
    with np.errstate(divide="ignore", invalid="ignore"):
        n_el = n_normal * D
        masked_sum = float((rowsum * nmf).sum())
        mean = masked_sum / n_el
        sum_sq_m = float((norms2 * nmf).sum())
        var = (sum_sq_m - 2.0 * mean * masked_sum + mean * mean * n_el) / (n_el - 1.0)
        sigma_new = 0.9 * float(running_sigma) + 0.1 * np.sqrt(var)
        m_adaptive = (MARGIN_BASE + LAMBDA_SIGMA * sigma_new
                      + LAMBDA_RESOLUTION * (1.0 - RESOLUTION_RATIO))
        dist = np.sqrt(np.maximum(dist_sq, 0.0))
        r_center = dist_sq * nmf
        r_margin = np.maximum(m_adaptive - dist, 0.0) * amf
        S_same = np.where(nmf, S0, S1)
        S_diff = np.where(nmf, S1, S0)
        pos_sum = S_same - ddiag
        neg_sum = S_diff
        n1 = B - n0
        cnt_pos = np.where(nmf, n0 - 1, n1 - 1)
        cnt_neg = np.where(nmf, n1, n0)
        has_both = (cnt_pos > 0) & (cnt_neg > 0)
        pos_safe = np.where(has_both, np.maximum(pos_sum, 1e-12), 1.0)
        den_safe = np.where(has_both, pos_sum + neg_sum + 1e-8, 1.0)
        r_con = np.where(has_both, -np.log(pos_safe / den_safe), 0.0)
        total = (ALPHA * r_center + BETA * r_margin + GAMMA * r_con).mean()
    return np.array(total, dtype=np.float32)


def kernel(features, labels, normal_center, running_sigma):
    features = np.asarray(features, dtype=np.float32)
    labels = np.asarray(labels, dtype=np.int32)
    normal_center = np.asarray(normal_center, dtype=np.float32)
    running_sigma = np.float32(np.asarray(running_sigma))
    Bq = features.shape[0]

    stats, order, n0, _res = run_device(features, labels)
    if float((np.asarray(normal_center, np.float64) ** 2).sum()) != 0.0:
        return _finalize_general_center(stats, order, n0, labels,
                                        normal_center, running_sigma, Bq,
                                        features)
    return finalize(stats, order, n0, labels, normal_center, running_sigma, Bq)


# revision 12
# speedup vs baseline: 1.2961x; 1.2961x over previous
"""MACCL loss kernel for Trainium2 (8 NeuronCores, SPMD data-parallel).

Strategy (v2)
-------------
The O(B^2 D) contrastive part dominates (B=8192, D=256).  The host does
the O(B*D) data prep that used to run on-device (and was the pipeline
bottleneck): permute rows label-0-first, compute row norms, quantize the
transposed features to fp8(e4m3) in the [K=128, 2, B] DoubleRow layout.
Each core then only runs the O(B^2) part:

  - 8x 1024-wide fp8 DoubleRow matmuls per 2048-column group: one
    instruction contracts the full K=256 (two k-tiles packed), so the
    PE does 0.5 cyc/row and the whole sim matrix costs ~14us/core.
  - ACT exp(scale_i * psum) in place, scale = r_i/T as a per-partition
    AP (the row normalization folds into the activation for free), with
    accum_out producing per-label-segment row sums (columns are
    label-sorted, segments are contiguous).
  - The diagonal term is reproduced bit-exactly by a DR matmul over the
    core's own (lhsT unscaled, rhs r_j-scaled) fp8 slices -- the same
    host arrays the main loop consumes -- so pos_sum = S_same - d
    cancels exactly on the host.

Per-core output: stats [128, 24] = {S0, S1, exp(diag)} x 8 row tiles.
Host finalizes in f64 (center/margin/sigma/log/mean) exactly mirroring
the reference formulas; norms/rowsums for the center and sigma terms
are host-side f64 (more accurate than the reference's own f32).

The operand quantization error only touches r_con, which is ~0.3% of
the total loss, so fp8 keeps the end-to-end error ~1e-5.
"""

import os
import sys

for _p in ("/root/.axon_site", "/root/.axon_site/_ro/trn_rl_repo",
           "/root/.axon_site/_ro/pypackages", "/opt/trn_rl_repo", "/opt/pypackages"):
    if os.path.isdir(_p) and _p not in sys.path:
        sys.path.append(_p)

import numpy as np
import ml_dtypes
from contextlib import ExitStack

import concourse.bass as bass
import concourse.bacc as bacc
import concourse.tile as tile
from concourse import mybir
from concourse.bass_utils import run_bass_kernel_spmd

F32 = mybir.dt.float32
BF16 = mybir.dt.bfloat16
F8 = mybir.dt.float8e4

P = 128
D = 256
B = 8192
NCORES = 8
BPC = B // NCORES
MROW = BPC // P
GW = 2048
NG = B // GW
TEMPERATURE = 0.07
MARGIN_BASE = 0.5
LAMBDA_SIGMA = 0.3
LAMBDA_RESOLUTION = 0.3
RESOLUTION_RATIO = 224.0 / 900.0
ALPHA, BETA, GAMMA = 1.0, 1.0, 0.5

# "f8dr" = fp8 DoubleRow (1 matmul per chunk, 0.5 cyc/row)
# "bf16" = bf16 with explicit k-loop (fallback)
MODE = os.environ.get("MACCL_MODE", "f8dr")
# moving-operand width per matmul (psum bank = 512 f32; DoubleRow N=1024
# fails the walrus s3d3_mm_num_elements ISA check, so 512 it is)
N_MM = int(os.environ.get("MACCL_N_MM", "512"))


def _segment_ranges(n0, gw=GW):
    """Column ranges per gw-wide group, split at the label boundary n0."""
    ranges = []
    for g in range(B // gw):
        lo, hi = g * gw, (g + 1) * gw
        cuts = sorted({lo, hi, min(max(n0, lo), hi)})
        for s, e in zip(cuts, cuts[1:]):
            if e > s:
                ranges.append((g, s, e, 0 if e <= n0 else 1))
    k0 = sum(1 for r in ranges if r[3] == 0)
    return ranges, k0


def build_program(n0, mode=MODE):
    use_f8 = mode == "f8dr"
    op_dt = F8 if use_f8 else BF16
    perf = mybir.MatmulPerfMode.DoubleRow if use_f8 else None

    ranges, k0 = _segment_ranges(n0)
    nslots = len(ranges)
    k1 = nslots - k0

    AX = mybir.AxisListType.X
    MUL = mybir.AluOpType.mult
    AF = mybir.ActivationFunctionType

    nc = bacc.Bacc("TRN2", target_bir_lowering=False, debug=False,
                   num_devices=NCORES)
    a8_d = nc.dram_tensor("a8", [P, 2, B], op_dt, kind="ExternalInput").ap()
    mm8_d = nc.dram_tensor("mm8", [P, 2, 2 * BPC], op_dt, kind="ExternalInput").ap()
    idr_d = nc.dram_tensor("idr", [P, P + MROW], F32, kind="ExternalInput").ap()
    stats_d = nc.dram_tensor("stats", [P, 3 * MROW], F32, kind="ExternalOutput").ap()

    with tile.TileContext(nc) as tc, ExitStack() as ctx:
        singles = ctx.enter_context(tc.tile_pool(name="singles", bufs=1))
        ps_pool = ctx.enter_context(tc.tile_pool(name="ps", bufs=1, space="PSUM"))

        a8_sb = singles.tile([P, 2, B], op_dt)
        mm8_sb = singles.tile([P, 2, 2 * BPC], op_dt)
        idr_sb = singles.tile([P, P + MROW], F32)
        stats_sb = singles.tile([P, 3 * MROW], F32)
        accs_t = singles.tile([P, MROW * nslots], F32)
        scr_t = singles.tile([P, P], F32)
        prime = singles.tile([P, 1], F32)

        m8_sb = mm8_sb[:, :, 0:BPC]
        ms8_sb = mm8_sb[:, :, BPC:2 * BPC]
        ident_t = idr_sb[:, 0:P]
        rot_sb = idr_sb[:, P:P + MROW]

        # Priming activation with no input deps (scale=0 ignores the garbage
        # read): hoists the ~1.5us ACT table load into the DMA lead-in.
        nc.scalar.activation(prime, prime, AF.Exp, scale=0.0)

        # packed DMAs for the small operands, then a8 chunk 0 (which gates
        # the first matmuls), then the rest
        nc.sync.dma_start(mm8_sb, mm8_d)
        nc.sync.dma_start(idr_sb, idr_d)
        for g in range(NG):
            nc.sync.dma_start(a8_sb[:, :, g * GW:(g + 1) * GW],
                              a8_d[:, :, g * GW:(g + 1) * GW])

        psA = ps_pool.tile([P, GW], F32, name="psA")
        psB = ps_pool.tile([P, GW], F32, name="psB")
        pss = [psA, psB]

        def mm(out_ap, lhsT, rhs_tile, c0, n):
            if use_f8:
                nc.tensor.matmul(out_ap, lhsT, rhs_tile[:, :, c0:c0 + n],
                                 start=True, stop=True, perf_mode=perf)
            else:
                for k in (0, 1):
                    nc.tensor.matmul(out_ap, lhsT[:, k, :],
                                     rhs_tile[:, k, c0:c0 + n],
                                     start=(k == 0), stop=(k == 1))

        def accs(m):
            return accs_t[:, m * nslots:(m + 1) * nslots]

        # ---- diag phase first: PE-light, queues 8 quick EXPs on ACT while
        # the a8 DMA streams in and the PE clock warms up ----
        nb = 0
        for m in range(MROW):
            psd = pss[nb]; nb ^= 1
            lhsT = m8_sb[:, :, m * P:(m + 1) * P]
            off = m * P
            c0 = (off // N_MM) * N_MM
            mm(psd[:, 0:N_MM], lhsT, ms8_sb, c0, N_MM)
            roff = off - c0
            nc.scalar.activation(psd[:, roff:roff + P], psd[:, roff:roff + P],
                                 AF.Exp, scale=rot_sb[:, m:m + 1])
            nc.vector.scalar_tensor_tensor(
                out=scr_t, in0=psd[:, roff:roff + P], scalar=1.0,
                in1=ident_t, op0=MUL, op1=MUL,
                accum_out=stats_sb[:, 2 * MROW + m:2 * MROW + m + 1])

        # ---- main loop: per column group, per own row tile ----
        for g in range(NG):
            for m in range(MROW):
                psg = pss[nb]; nb ^= 1
                lhsT = m8_sb[:, :, m * P:(m + 1) * P]
                for s in range(GW // N_MM):
                    mm(psg[:, s * N_MM:(s + 1) * N_MM], lhsT, a8_sb,
                       g * GW + s * N_MM, N_MM)
                for slot, (gg, s, e, _lab) in enumerate(ranges):
                    if gg != g:
                        continue
                    rs, re = s - g * GW, e - g * GW
                    nc.scalar.activation(
                        psg[:, rs:re], psg[:, rs:re], AF.Exp,
                        scale=rot_sb[:, m:m + 1],
                        accum_out=accs(m)[:, slot:slot + 1])

        # ---- per-row-tile S0/S1 ----
        for m in range(MROW):
            s0 = stats_sb[:, m:m + 1]
            s1 = stats_sb[:, MROW + m:MROW + m + 1]
            if k0 > 0:
                nc.vector.reduce_sum(s0, accs(m)[:, 0:k0], axis=AX)
            else:
                nc.vector.memset(s0, 0.0)
            if k1 > 0:
                nc.vector.reduce_sum(s1, accs(m)[:, k0:nslots], axis=AX)
            else:
                nc.vector.memset(s1, 0.0)

        nc.sync.dma_start(stats_d, stats_sb)

    nc.compile()
    return nc


_PROGRAM_CACHE = {}


def _get_program(n0):
    key = (n0, MODE, N_MM)
    if key not in _PROGRAM_CACHE:
        _PROGRAM_CACHE[key] = build_program(n0)
    return _PROGRAM_CACHE[key]


def run_device(features, labels, trace=False):
    """Host prep + 8-core device run.  Returns (stats dict aligned to the
    label-sorted permutation, permutation order, n0, raw results)."""
    Bq, d = features.shape
    assert d == D and Bq == B

    order = np.argsort(labels, kind="stable")
    n0 = int((labels == 0).sum())
    fp = np.ascontiguousarray(features[order]).astype(np.float32, copy=False)

    # host-side O(B*D) prep
    fp64 = fp.astype(np.float64)
    norms2 = (fp64 * fp64).sum(axis=1)                  # [B]
    rowsum = fp64.sum(axis=1)                           # [B]
    r = 1.0 / np.maximum(np.sqrt(norms2), 1e-12)        # [B]
    r32 = r.astype(np.float32)

    op_np = ml_dtypes.float8_e4m3 if MODE == "f8dr" else ml_dtypes.bfloat16
    # [K=128, 2, B] DoubleRow layout: D index = ktile*128 + partition
    fT = np.ascontiguousarray(fp.T).reshape(2, P, B).transpose(1, 0, 2)
    m8_full = np.ascontiguousarray(fT).astype(op_np)                 # unscaled
    a8 = np.ascontiguousarray(fT * r32[None, None, :]).astype(op_np)  # scaled

    rot_full = (r32 / np.float32(TEMPERATURE)).astype(np.float32)
    ident = np.eye(P, dtype=np.float32)

    nc = _get_program(n0)
    in_maps = []
    for c in range(NCORES):
        sl = slice(c * BPC, (c + 1) * BPC)
        mm8 = np.concatenate([m8_full[:, :, sl], a8[:, :, sl]], axis=2)
        idr = np.concatenate(
            [ident, rot_full[sl].reshape(MROW, P).T], axis=1)
        in_maps.append({"a8": a8,
                        "mm8": np.ascontiguousarray(mm8),
                        "idr": np.ascontiguousarray(idr)})
    res = run_bass_kernel_spmd(nc, in_maps, list(range(NCORES)), trace=trace)

    parts = []
    for c in range(NCORES):
        st = res.results[c]["stats"]          # [128, 3*MROW]
        arr = st.reshape(P, 3, MROW).transpose(1, 2, 0).reshape(3, BPC)
        parts.append(arr)
    full = np.concatenate(parts, axis=1)      # [3, B] in permuted row order
    stats = {"norms2": norms2, "rowsum": rowsum,
             "S0": full[0], "S1": full[1], "d": full[2]}
    return stats, order, n0, res


def finalize(stats, order, n0, labels, normal_center, running_sigma, B):
    """Host O(B) finalization mirroring the reference formulas (float64)."""
    labels_p = labels[order]
    nmf = (labels_p == 0)
    amf = (labels_p == 1)
    norms2 = stats["norms2"].astype(np.float64)
    rowsum = stats["rowsum"].astype(np.float64)
    S0 = stats["S0"].astype(np.float64)
    S1 = stats["S1"].astype(np.float64)
    ddiag = stats["d"].astype(np.float64)

    c = np.asarray(normal_center, dtype=np.float64)
    csq = float((c * c).sum())
    if csq != 0.0:
        raise NotImplementedError  # caller routes to the general-center path
    dist_sq = norms2  # center == 0
    n_normal = float(nmf.sum())

    with np.errstate(divide="ignore", invalid="ignore"):
        n_el = n_normal * D
        masked_sum = float((rowsum * nmf).sum())
        mean = masked_sum / n_el
        sum_sq_m = float((norms2 * nmf).sum())
        var = (sum_sq_m - 2.0 * mean * masked_sum + mean * mean * n_el) / (n_el - 1.0)
        sigma_new = 0.9 * float(running_sigma) + 0.1 * np.sqrt(var)

        m_adaptive = (MARGIN_BASE + LAMBDA_SIGMA * sigma_new
                      + LAMBDA_RESOLUTION * (1.0 - RESOLUTION_RATIO))
        dist = np.sqrt(np.maximum(dist_sq, 0.0))
        r_center = dist_sq * nmf
        r_margin = np.maximum(m_adaptive - dist, 0.0) * amf

        S_same = np.where(nmf, S0, S1)
        S_diff = np.where(nmf, S1, S0)
        pos_sum = S_same - ddiag
        neg_sum = S_diff
        n1 = B - n0
        cnt_pos = np.where(nmf, n0 - 1, n1 - 1)
        cnt_neg = np.where(nmf, n1, n0)
        has_both = (cnt_pos > 0) & (cnt_neg > 0)
        pos_safe = np.where(has_both, np.maximum(pos_sum, 1e-12), 1.0)
        den_safe = np.where(has_both, pos_sum + neg_sum + 1e-8, 1.0)
        r_con = np.where(has_both, -np.log(pos_safe / den_safe), 0.0)

        raw_total = ALPHA * r_center + BETA * r_margin + GAMMA * r_con
        total = raw_total.mean()
    return np.array(total, dtype=np.float32)


def _finalize_general_center(stats, order, n0, labels, normal_center,
                             running_sigma, B, features):
    """Fallback for a nonzero normal_center (not hit for spec inputs)."""
    labels_p = labels[order]
    fp = features[order].astype(np.float64)
    c = np.asarray(normal_center, dtype=np.float64)
    qc = fp @ c
    norms2 = stats["norms2"].astype(np.float64)
    dist_sq = norms2 - 2.0 * qc + float((c * c).sum())
    nmf = (labels_p == 0)
    amf = (labels_p == 1)
    rowsum = stats["rowsum"].astype(np.float64)
    S0 = stats["S0"].astype(np.float64)
    S1 = stats["S1"].astype(np.float64)
    ddiag = stats["d"].astype(np.float64)
    n_normal = float(nmf.sum())
    with np.errstate(divide="ignore", invalid="ignore"):
        n_el = n_normal * D
        masked_sum = float((rowsum * nmf).sum())
        mean = masked_sum / n_el
        sum_sq_m = float((norms2 * nmf).sum())
        var = (sum_sq_m - 2.0 * mean * masked_sum + mean * mean * n_el) / (n_el - 1.0)
        sigma_new = 0.9 * float(running_sigma) + 0.1 * np.sqrt(var)
        m_adaptive = (MARGIN_BASE + LAMBDA_SIGMA * sigma_new
                      + LAMBDA_RESOLUTION * (1.0 - RESOLUTION_RATIO))
        dist = np.sqrt(np.maximum(dist_sq, 0.0))
        r_center = dist_sq * nmf
        r_margin = np.maximum(m_adaptive - dist, 0.0) * amf
        S_same = np.where(nmf, S0, S1)
        S_diff = np.where(nmf, S1, S0)
        pos_sum = S_same - ddiag
        neg_sum = S_diff
        n1 = B - n0
        cnt_pos = np.where(nmf, n0 - 1, n1 - 1)
        cnt_neg = np.where(nmf, n1, n0)
        has_both = (cnt_pos > 0) & (cnt_neg > 0)
        pos_safe = np.where(has_both, np.maximum(pos_sum, 1e-12), 1.0)
        den_safe = np.where(has_both, pos_sum + neg_sum + 1e-8, 1.0)
        r_con = np.where(has_both, -np.log(pos_safe / den_safe), 0.0)
        total = (ALPHA * r_center + BETA * r_margin + GAMMA * r_con).mean()
    return np.array(total, dtype=np.float32)


def kernel(features, labels, normal_center, running_sigma):
    features = np.asarray(features, dtype=np.float32)
    labels = np.asarray(labels, dtype=np.int32)
    normal_center = np.asarray(normal_center, dtype=np.float32)
    running_sigma = np.float32(np.asarray(running_sigma))
    Bq = features.shape[0]

    stats, order, n0, _res = run_device(features, labels)
    if float((np.asarray(normal_center, np.float64) ** 2).sum()) != 0.0:
        return _finalize_general_center(stats, order, n0, labels,
                                        normal_center, running_sigma, Bq,
                                        features)
    return finalize(stats, order, n0, labels, normal_center, running_sigma, Bq)
